# revision 41
# baseline (speedup 1.0000x reference)
"""Two-layer GCN (GCNConv x2 + log_softmax) on 8 Trainium2 NeuronCores.

Strategy (graph/data parallel, nodes sharded 8 ways):
  - Norm factors dinv[src]*dinv[dst] factor into a pre-scale of the gather
    table rows and a post-scale of the aggregated output, so aggregation is a
    pure unweighted segment-sum of gathered 256B rows.
  - Layer tables (T1 = (dinv*x)@W1, T3 = allgather((dinv*relu(out1))@W2)) are
    node-major [N, 128] bf16 in DRAM (values in the first 64 cols, upper half
    never read -- dma_gather needs 256B rows); per-edge rows are fetched with
    gpsimd.dma_gather (int16 indices -> 4 windows of 25k rows,
    single_packet=False: single-packet mode crashes above ~1024 rows/call).
  - Segment-sum via selection-matrix matmuls: for each column of 128 messages
    belonging to one 128-node dst group, S[m, r] = (localid[m] == r) built
    on-device with a broadcast is_equal against an iota row; TensorE
    accumulates norm'd messages into the group's PSUM accumulator.
  - Layer 1 runs feature-major (M as lhsT) so T2^T feeds the W2 matmul
    directly; layer 2 runs node-major (S as lhsT) so log_softmax reduces
    along the free axis.
  - One AllGather between the layers exchanges the [12500, 64] f32 slices.
"""

import math
import os
from contextlib import ExitStack
from dataclasses import dataclass

import numpy as np
import ml_dtypes

import concourse.bass as bass
import concourse.tile as tile
from concourse import bacc, mybir
from concourse.bass_utils import run_bass_kernel_spmd

F32 = mybir.dt.float32
BF16 = mybir.dt.bfloat16
I16 = mybir.dt.int16
AF = mybir.ActivationFunctionType
ALU = mybir.AluOpType


@dataclass
class Cfg:
    n: int = 100000        # nodes
    nin: int = 128         # input features
    hid: int = 64          # hidden features (= table row width, 256B f32)
    outf: int = 40         # output features
    ncores: int = 8
    nwin: int = 4          # gather-table windows (int16 idx range)
    g: int = 128           # dst group size
    chunk_g: int = 12      # groups per PSUM chunk
    sub: int = 32          # columns per S-build/cast sub-slab
    xchunk: int = 2048     # nodes per T1-build matmul chunk

    @property
    def per(self):
        return self.n // self.ncores

    @property
    def win(self):
        return self.n // self.nwin

    @property
    def ng(self):
        return math.ceil(self.per / self.g)

    @property
    def perp(self):
        return self.ng * self.g


# ---------------------------------------------------------------- host side


def _preprocess(x, edge_index, W1, b1, W2, b2, cfg: Cfg):
    n, per, g, win = cfg.n, cfg.per, cfg.g, cfg.win
    nc_, ng, nwin = cfg.ncores, cfg.ng, cfg.nwin

    loops = np.arange(n, dtype=np.int64)
    src = np.concatenate([edge_index[0].astype(np.int64), loops])
    dst = np.concatenate([edge_index[1].astype(np.int64), loops])

    deg = np.bincount(dst, minlength=n).astype(np.float64)
    dinv = np.where(deg > 0, 1.0 / np.sqrt(deg), 0.0).astype(np.float32)

    # table pre-scale folded in on host
    xs = (x * dinv[:, None]).astype(np.float32)
    xsT = np.ascontiguousarray(xs.T).astype(ml_dtypes.bfloat16)  # [nin, n]

    # ---- per-core edge buckets ----
    core = dst // per
    gidx = (dst % per) // g
    widx = src // win
    lid = (dst % per) % g

    # counts[c, g, w]
    counts = np.zeros((nc_, ng, nwin), dtype=np.int64)
    np.add.at(counts, (core, gidx, widx), 1)
    ncols_gw = np.ceil(counts / g).max(axis=0).astype(np.int64)  # [ng, nwin]

    # chunk layout: for K: for w: for g in K: ncols_gw[g, w] columns
    chunks = [
        list(range(k0, min(k0 + cfg.chunk_g, ng))) for k0 in range(0, ng, cfg.chunk_g)
    ]
    # stream metadata
    call_meta = []  # per (K, w): (slot_off, n_slots, [(gid, ncols), ...])
    region_off = {}  # (g, w) -> slot offset of its region
    off = 0
    for K in chunks:
        for w in range(nwin):
            groups = [(gg, int(ncols_gw[gg, w])) for gg in K if ncols_gw[gg, w] > 0]
            sl0 = off
        # register region offsets
            for gg, ncol in groups:
                region_off[(gg, w)] = off
                off += ncol * g
            call_meta.append((sl0, off - sl0, groups))
    tot_slots = off
    tot_cols = tot_slots // g

    # ---- per-core idx / localid arrays ----
    order = np.lexsort((src, widx, gidx, core))  # sort by core, g, w, src
    src_s, core_s = src[order], core[order]
    g_s, w_s, lid_s = gidx[order], widx[order], lid[order]

    idx_all = np.zeros((nc_, tot_slots), dtype=np.int16)
    lid_all = np.full((nc_, tot_slots), 255.0, dtype=np.float32)
    for c in range(nc_):
        m = core_s == c
        sc, gc, wc, lc = src_s[m], g_s[m], w_s[m], lid_s[m]
        # slot position: region_off[(g, w)] + rank within (g, w)
        # compute rank within each (g,w) run (data is sorted by (g,w))
        key = gc * nwin + wc
        # run-start indices
        change = np.r_[True, key[1:] != key[:-1]]
        run_id = np.cumsum(change) - 1
        run_start = np.flatnonzero(change)
        rank = np.arange(len(key)) - run_start[run_id]
        base = np.array([region_off[(gg, ww)] for gg, ww in zip(gc[change], wc[change])])
        slot = base[run_id] + rank
        idx_all[c, slot] = (sc - wc * win).astype(np.int16)
        lid_all[c, slot] = lc

    # wrap idx into [128, tot_slots//16] (16-partition wrap, replicated x8)
    idx_wrap = np.zeros((nc_, 128, tot_slots // 16), dtype=np.int16)
    lid_cols = np.zeros((nc_, 128, tot_cols), dtype=ml_dtypes.bfloat16)
    for c in range(nc_):
        wrapped = idx_all[c].reshape(-1, 16).T  # [16, S/16]
        idx_wrap[c] = np.tile(wrapped, (8, 1))
        lid_cols[c] = lid_all[c].reshape(tot_cols, g).T.astype(ml_dtypes.bfloat16)

    # ---- small constants ----
    iota = np.tile(np.arange(g, dtype=np.float32), (g, 1)).astype(ml_dtypes.bfloat16)
    W1bf = W1.astype(ml_dtypes.bfloat16)  # [nin, hid]
    W2p = np.zeros((cfg.hid, cfg.hid), dtype=np.float32)
    W2p[:, : cfg.outf] = W2
    W2bf = W2p.astype(ml_dtypes.bfloat16)
    b1col = b1.reshape(cfg.hid, 1).astype(np.float32)
    b2bc = np.zeros((128, cfg.hid), dtype=np.float32)
    b2bc[:, : cfg.outf] = b2[None, :]

    dinv_bc = np.zeros((nc_, cfg.hid, cfg.perp), dtype=np.float32)
    dinv_col = np.zeros((nc_, 128, ng), dtype=np.float32)
    for c in range(nc_):
        dslice = np.concatenate(
            [dinv[c * per : (c + 1) * per], np.ones(cfg.perp - per, np.float32)]
        )
        dinv_bc[c] = np.tile(dslice, (cfg.hid, 1))
        dinv_col[c] = dslice.reshape(ng, g).T

    in_maps = []
    for c in range(nc_):
        in_maps.append(
            {
                "xsT": np.asarray(xsT),
                "W1bf": np.asarray(W1bf),
                "W2bf": np.asarray(W2bf),
                "b1col": b1col,
                "b2bc": b2bc,
                "iota": np.asarray(iota),
                "idx": idx_wrap[c],
                "lid": np.asarray(lid_cols[c]),
                "dinv_bc": dinv_bc[c],
                "dinv_col": dinv_col[c],
            }
        )

    sched = dict(call_meta=call_meta, chunks=chunks, ncols_gw=ncols_gw,
                 tot_slots=tot_slots, tot_cols=tot_cols)
    return in_maps, sched


# ---------------------------------------------------------------- device side


def _build(cfg: Cfg, sched) -> bacc.Bacc:
    n, hid, g, nwin, win = cfg.n, cfg.hid, cfg.g, cfg.nwin, cfg.win
    ng, per, perp = cfg.ng, cfg.per, cfg.perp
    call_meta, chunks = sched["call_meta"], sched["chunks"]
    tot_slots, tot_cols = sched["tot_slots"], sched["tot_cols"]
    ncols_gw = sched["ncols_gw"]
    max_call_cols = max((m[1] // g for m in call_meta), default=1)

    nc = bacc.Bacc("TRN2", target_bir_lowering=False, debug=False,
                   num_devices=cfg.ncores)

    xsT = nc.dram_tensor("xsT", [cfg.nin, n], BF16, kind="ExternalInput").ap()
    W1bf = nc.dram_tensor("W1bf", [cfg.nin, hid], BF16, kind="ExternalInput").ap()
    W2bf = nc.dram_tensor("W2bf", [hid, hid], BF16, kind="ExternalInput").ap()
    b1col = nc.dram_tensor("b1col", [hid, 1], F32, kind="ExternalInput").ap()
    b2bc = nc.dram_tensor("b2bc", [128, hid], F32, kind="ExternalInput").ap()
    iota_d = nc.dram_tensor("iota", [g, g], BF16, kind="ExternalInput").ap()
    idx_d = nc.dram_tensor("idx", [128, tot_slots // 16], I16, kind="ExternalInput").ap()
    lid_d = nc.dram_tensor("lid", [128, tot_cols], BF16, kind="ExternalInput").ap()
    dinv_bc_d = nc.dram_tensor("dinv_bc", [hid, perp], F32, kind="ExternalInput").ap()
    dinv_col_d = nc.dram_tensor("dinv_col", [128, ng], F32, kind="ExternalInput").ap()

    out_d = nc.dram_tensor("out", [per, cfg.outf], F32, kind="ExternalOutput").ap()

    # Tables are [n, 2*hid] bf16: first hid cols hold values, upper half is
    # never read (dma_gather needs 256B rows; the pad halves are garbage).
    T1 = nc.dram_tensor("T1", [n, 2 * hid], BF16).ap()
    # Compact AllGather: ship only the outf real output columns, then expand
    # locally into the 256B-row gather table (pad columns are never read).
    of_ = cfg.outf
    h2c_b = nc.dram_tensor("h2c_b", [per, of_], BF16).ap()         # AG input bounce
    T3c = nc.dram_tensor("T3c", [n, of_], BF16, addr_space="Shared").ap()
    T3 = nc.dram_tensor("T3", [n, 2 * hid], BF16).ap()             # expanded table

    from concourse import library_config

    with tile.TileContext(nc) as tc, ExitStack() as ctx:
        nc.gpsimd.load_library(library_config.mlp)

        consts = ctx.enter_context(tc.tile_pool(name="consts", bufs=1))
        sb = ctx.enter_context(tc.tile_pool(name="sb", bufs=2))
        subp = ctx.enter_context(tc.tile_pool(name="subp", bufs=3))
        eptmp = ctx.enter_context(tc.tile_pool(name="eptmp", bufs=4))
        psum_bank = ctx.enter_context(tc.tile_pool(name="psumb", bufs=6, space="PSUM"))
        psum_mm = ctx.enter_context(tc.tile_pool(name="psummm", bufs=2, space="PSUM"))

        # resident constants
        w1_t = consts.tile([cfg.nin, hid], BF16)
        nc.sync.dma_start(w1_t[:], W1bf[:, :])
        w2_t = consts.tile([hid, hid], BF16)
        nc.sync.dma_start(w2_t[:], W2bf[:, :])
        b1_t = consts.tile([hid, 1], F32)
        nc.sync.dma_start(b1_t[:], b1col[:, :])
        b2_t = consts.tile([128, hid], F32)
        nc.sync.dma_start(b2_t[:], b2bc[:, :])
        iota_t = consts.tile([g, g], BF16)
        nc.sync.dma_start(iota_t[:], iota_d[:, :])
        # dense repeated iota [128, sub*g] so the S-build tensor_tensor has a
        # dense stride-1 first operand (DVE 2x eligibility)
        iota_rep = consts.tile([g, cfg.sub * g], BF16)
        for _s in range(cfg.sub):
            nc.vector.tensor_copy(iota_rep[:, _s * g : (_s + 1) * g], iota_t[:])
        dinvbc_t = consts.tile([hid, perp], F32)
        nc.sync.dma_start(dinvbc_t[:], dinv_bc_d[:, :])
        dinvcol_t = consts.tile([128, ng], F32)
        nc.sync.dma_start(dinvcol_t[:], dinv_col_d[:, :])
        t2t = consts.tile([hid, perp], BF16)  # T2^T staging (layer-1 output)
        # resident index/localid streams (shared by both layers)
        idx_t = consts.tile([128, tot_slots // 16], I16)
        nc.sync.dma_start(idx_t[:], idx_d[:, :])
        lid_t = consts.tile([128, tot_cols], BF16)
        nc.sync.dma_start(lid_t[:], lid_d[:, :])

        # -------------------------------------------------- T1 build
        # xchunk nodes per iteration: one input DMA, xchunk/128 matmuls into
        # psum banks of 8 x [128, 64], bank-wise ACT evictions, one fused 3D
        # output DMA (DRAM view [nsub, 128, hid] <- SBUF [128, nsub, hid]).
        xc = cfg.xchunk
        for ch0 in range(0, n, xc):
            cols = min(xc, n - ch0)
            nsub = math.ceil(cols / 128)
            xt = sb.tile([cfg.nin, xc], BF16, tag="xt")
            nc.sync.dma_start(xt[:, :cols], xsT[:, ch0 : ch0 + cols])
            st = sb.tile([128, xc // 128, hid], BF16, tag="t1s")
            for b0 in range(0, nsub, 8):
                bn = min(8, nsub - b0)
                pt = psum_mm.tile([128, 512], F32, tag="t1p", name="pt")
                for qi in range(bn):
                    q = b0 + qi
                    qc = min(128, cols - q * 128)
                    nc.tensor.matmul(
                        out=pt[:qc, qi * hid : qi * hid + hid],
                        lhsT=xt[:, q * 128 : q * 128 + qc],
                        rhs=w1_t[:],
                        start=True,
                        stop=True,
                    )
                nc.scalar.copy(
                    st[:, b0 : b0 + bn, :].rearrange("p q h -> p (q h)"),
                    pt[:, : bn * hid],
                )
            # fused transposed store into the value halves of T1 rows
            if cols % 128 == 0:
                dram_view = T1[ch0 : ch0 + cols, :hid].rearrange(
                    "(q p) h -> p q h", p=128
                )
                nc.sync.dma_start(dram_view, st[:, :nsub, :])
            else:
                for q in range(nsub):
                    qc = min(128, cols - q * 128)
                    nc.sync.dma_start(
                        T1[ch0 + q * 128 : ch0 + q * 128 + qc, :hid],
                        st[:qc, q, :],
                    )

        # -------------------------------------------------- aggregation layers
        def agg_layer(layer: int, table_ap):
            """layer 1: feature-major accum [hid, 128]; writes t2t + h2s.
            layer 2: node-major accum [128, hid]; writes log_softmax to out."""
            gper = 4 if layer == 1 else 8  # accumulator regions per PSUM bank
            ci = 0
            for K in chunks:
                # per-group accumulator sub-regions inside full-bank tiles
                nbank = math.ceil(len(K) / gper)
                banks = [
                    psum_bank.tile([128, 512], F32, tag="acc", name=f"acc{layer}")
                    for _ in range(nbank)
                ]

                def acc_ap(j):
                    b = banks[j // gper]
                    if layer == 1:
                        r = j % 4
                        return b[0:64, r * 128 : r * 128 + 128]
                    r = j % 8
                    return b[:, r * 64 : r * 64 + 64]

                # start/stop are BANK-granular: start=True clears has_written
                # for the whole bank, so only the first matmul into each bank
                # tile uses start=True and only the last uses stop=True;
                # per-region init relies on per-element overwrite semantics.
                tot_bank = [0] * nbank
                for j, gg in enumerate(K):
                    tot_bank[j // gper] += int(
                        sum(ncols_gw[gg, w] for w in range(nwin))
                    )
                emitted_bank = [0] * nbank
                for w in range(nwin):
                    sl0, nsl, groups = call_meta[ci]
                    ci += 1
                    if nsl == 0:
                        continue
                    cols = nsl // g
                    mt = sb.tile([128, max_call_cols, 2 * hid], BF16, tag="m")
                    nc.gpsimd.dma_gather(
                        mt[:, :cols, :],
                        table_ap[w * win : (w + 1) * win, :],
                        idx_t[:, sl0 // 16 : (sl0 + nsl) // 16],
                        nsl,
                        nsl,
                        2 * hid,
                        single_packet=False,
                    )
                    # column -> (group-in-K index) map
                    colg = []
                    for gg, ncol in groups:
                        colg += [K.index(gg)] * ncol
                    for s0 in range(0, cols, cfg.sub):
                        sc = min(cfg.sub, cols - s0)
                        c0 = sl0 // g + s0
                        st_ = subp.tile([128, cfg.sub * g], BF16, tag="sel")
                        nc.vector.tensor_tensor(
                            out=st_[:, : sc * g].rearrange("p (c r) -> p c r", r=g),
                            in0=iota_rep[:, : sc * g].rearrange(
                                "p (c r) -> p c r", r=g
                            ),
                            in1=lid_t[:, c0 : c0 + sc].to_broadcast([128, sc, g]),
                            op=ALU.is_equal,
                        )
                        for j in range(sc):
                            gj = colg[s0 + j]
                            b = gj // gper
                            first = emitted_bank[b] == 0
                            emitted_bank[b] += 1
                            last = emitted_bank[b] == tot_bank[b]
                            if layer == 1:
                                nc.tensor.matmul(
                                    out=acc_ap(gj),
                                    lhsT=mt[:, s0 + j, :hid],
                                    rhs=st_[:, j * g : (j + 1) * g],
                                    start=first,
                                    stop=last,
                                )
                            else:
                                nc.tensor.matmul(
                                    out=acc_ap(gj),
                                    lhsT=st_[:, j * g : (j + 1) * g],
                                    rhs=mt[:, s0 + j, :hid],
                                    start=first,
                                    stop=last,
                                )
                # epilogues for chunk K
                if layer == 2:
                    ot_stage = sb.tile(
                        [128, cfg.chunk_g, cfg.outf], F32, tag="otst", name="ot_stage"
                    )
                for j, gg in enumerate(K):
                    rows = min(g, per - gg * g)  # real dst rows in group
                    if layer == 1:
                        dslice = dinvbc_t[:, gg * g : gg * g + g]
                        t1_ = eptmp.tile([hid, g], F32, tag="ep1")
                        nc.vector.tensor_mul(t1_[:], acc_ap(j)[:], dslice[:])
                        t2_ = eptmp.tile([hid, g], F32, tag="ep2")
                        nc.scalar.activation(t2_[:], t1_[:], AF.Relu, bias=b1_t[:, :1])
                        nc.vector.tensor_mul(
                            t2t[:, gg * g : gg * g + g], t2_[:], dslice[:]
                        )
                    else:
                        t1_ = eptmp.tile([128, hid], F32, tag="ep1")
                        nc.vector.tensor_scalar(
                            t1_[:], acc_ap(j)[:], dinvcol_t[:, gg : gg + 1], None,
                            ALU.mult,
                        )
                        t2_ = eptmp.tile([128, hid], F32, tag="ep2")
                        nc.vector.tensor_add(t2_[:], t1_[:], b2_t[:])
                        of = cfg.outf
                        nmax = eptmp.tile([128, 1], F32, tag="nmax")
                        nc.vector.tensor_reduce(
                            nmax[:], t2_[:, :of], mybir.AxisListType.X, ALU.max,
                            negate=True,
                        )
                        ex = eptmp.tile([128, of], F32, tag="ex")
                        nc.scalar.activation(ex[:], t2_[:, :of], AF.Exp, bias=nmax[:, :1])
                        sm = eptmp.tile([128, 1], F32, tag="sm")
                        nc.vector.tensor_reduce(
                            sm[:], ex[:], mybir.AxisListType.X, ALU.add
                        )
                        ls = eptmp.tile([128, 1], F32, tag="ls")
                        nc.scalar.activation(ls[:], sm[:], AF.Ln)
                        nc.vector.tensor_scalar(
                            ot_stage[:, j, :], t2_[:, :of], nmax[:, :1], ls[:, :1],
                            ALU.add, ALU.subtract,
                        )
                if layer == 2:
                    # one fused store for the chunk's full groups, small store
                    # for a trailing partial group
                    nfull = sum(1 for gg in K if per - gg * g >= g)
                    r0 = K[0] * g
                    if nfull:
                        nc.sync.dma_start(
                            out_d[r0 : r0 + nfull * g, :].rearrange(
                                "(q p) f -> p q f", p=128
                            ),
                            ot_stage[:, :nfull, :],
                        )
                    for j, gg in enumerate(K):
                        rows = per - gg * g
                        if rows < g:
                            nc.sync.dma_start(
                                out_d[gg * g : gg * g + rows, :],
                                ot_stage[:rows, j, :],
                            )

        _phases = int(os.environ.get("GCN_PHASES", "4"))  # 1=t1 2=+l1 3=+ag 4=all
        if _phases >= 2:
            agg_layer(1, T1)

        # -------------------------------------------------- W2 matmul + AllGather
        if _phases >= 3:
            for g0 in range(0, ng, 8):
                gn = min(8, ng - g0)
                pw = psum_mm.tile([128, 512], F32, tag="t1p", name="pw")
                for qi in range(gn):
                    gg = g0 + qi
                    nc.tensor.matmul(
                        out=pw[:, qi * hid : qi * hid + hid],
                        lhsT=t2t[:, gg * g : gg * g + g],
                        rhs=w2_t[:],
                        start=True,
                        stop=True,
                    )
                hw = sb.tile([128, 8, hid], BF16, tag="h2t")
                nc.scalar.copy(
                    hw[:, :gn, :].rearrange("p q h -> p (q h)"), pw[:, : gn * hid]
                )
                nfull = sum(1 for gg in range(g0, g0 + gn) if per - gg * g >= g)
                if nfull:
                    nc.sync.dma_start(
                        h2c_b[g0 * g : (g0 + nfull) * g, :].rearrange(
                            "(q p) h -> p q h", p=128
                        ),
                        hw[:, :nfull, :of_],
                    )
                for qi in range(gn):
                    gg = g0 + qi
                    rows = per - gg * g
                    if rows < g:
                        nc.sync.dma_start(
                            h2c_b[gg * g : gg * g + rows, :], hw[:rows, qi, :of_]
                        )

        if _phases >= 3 and not os.environ.get("GCN_NO_COLL"):
            nc.gpsimd.collective_compute(
                "AllGather",
                ALU.bypass,
                replica_groups=[list(range(cfg.ncores))],
                ins=[h2c_b.opt()],
                outs=[T3c.opt()],
            )
            # expand compact rows into the 256B-row gather table; the pad
            # columns of T3 stay uninitialized and are never read.
            for r0 in range(0, n, per):
                nc.sync.dma_start(T3[r0 : r0 + per, :of_], T3c[r0 : r0 + per, :])

        if _phases >= 4:
            agg_layer(2, T3)

    nc.compile()
    return nc


# ---------------------------------------------------------------- entry


def kernel(x, edge_index, W1, b1, W2, b2, cfg: Cfg | None = None, _run=None):
    cfg = cfg or Cfg()
    in_maps, sched = _preprocess(
        np.asarray(x), np.asarray(edge_index), np.asarray(W1), np.asarray(b1),
        np.asarray(W2), np.asarray(b2), cfg
    )
    nc = _build(cfg, sched)
    if _run is not None:  # test hook (e.g. simulator)
        results = _run(nc, in_maps)
    else:
        results = run_bass_kernel_spmd(
            nc, in_maps, core_ids=list(range(cfg.ncores))
        ).results
    out = np.concatenate([results[c]["out"] for c in range(cfg.ncores)], axis=0)
    return out.astype(np.float32)


# revision 43
# speedup vs baseline: 1.0147x; 1.0147x over previous
"""Two-layer GCN (GCNConv x2 + log_softmax) on 8 Trainium2 NeuronCores.

Strategy (graph/data parallel, nodes sharded 8 ways):
  - Norm factors dinv[src]*dinv[dst] factor into a pre-scale of the gather
    table rows and a post-scale of the aggregated output, so aggregation is a
    pure unweighted segment-sum of gathered 256B rows.
  - Layer tables (T1 = (dinv*x)@W1, T3 = allgather((dinv*relu(out1))@W2)) are
    node-major [N, 128] bf16 in DRAM (values in the first 64 cols, upper half
    never read -- dma_gather needs 256B rows); per-edge rows are fetched with
    gpsimd.dma_gather (int16 indices -> 4 windows of 25k rows,
    single_packet=False: single-packet mode crashes above ~1024 rows/call).
  - Segment-sum via selection-matrix matmuls: for each column of 128 messages
    belonging to one 128-node dst group, S[m, r] = (localid[m] == r) built
    on-device with a broadcast is_equal against an iota row; TensorE
    accumulates norm'd messages into the group's PSUM accumulator.
  - Layer 1 runs feature-major (M as lhsT) so T2^T feeds the W2 matmul
    directly; layer 2 runs node-major (S as lhsT) so log_softmax reduces
    along the free axis.
  - One AllGather between the layers exchanges the [12500, 64] f32 slices.
"""

import math
import os
from contextlib import ExitStack
from dataclasses import dataclass

import numpy as np
import ml_dtypes

import concourse.bass as bass
import concourse.tile as tile
from concourse import bacc, mybir
from concourse.bass_utils import run_bass_kernel_spmd

F32 = mybir.dt.float32
BF16 = mybir.dt.bfloat16
I16 = mybir.dt.int16
AF = mybir.ActivationFunctionType
ALU = mybir.AluOpType


@dataclass
class Cfg:
    n: int = 100000        # nodes
    nin: int = 128         # input features
    hid: int = 64          # hidden features (= table row width, 256B f32)
    outf: int = 40         # output features
    ncores: int = 8
    nwin: int = 4          # gather-table windows (int16 idx range)
    g: int = 128           # dst group size
    chunk_g: int = 12      # groups per PSUM chunk
    sub: int = 32          # columns per S-build/cast sub-slab
    xchunk: int = 2048     # nodes per T1-build matmul chunk

    @property
    def per(self):
        return self.n // self.ncores

    @property
    def win(self):
        return self.n // self.nwin

    @property
    def ng(self):
        return math.ceil(self.per / self.g)

    @property
    def perp(self):
        return self.ng * self.g


# ---------------------------------------------------------------- host side


def _preprocess(x, edge_index, W1, b1, W2, b2, cfg: Cfg):
    n, per, g, win = cfg.n, cfg.per, cfg.g, cfg.win
    nc_, ng, nwin = cfg.ncores, cfg.ng, cfg.nwin

    loops = np.arange(n, dtype=np.int64)
    src = np.concatenate([edge_index[0].astype(np.int64), loops])
    dst = np.concatenate([edge_index[1].astype(np.int64), loops])

    deg = np.bincount(dst, minlength=n).astype(np.float64)
    dinv = np.where(deg > 0, 1.0 / np.sqrt(deg), 0.0).astype(np.float32)

    # table pre-scale folded in on host
    xs = (x * dinv[:, None]).astype(np.float32)
    xsT = np.ascontiguousarray(xs.T).astype(ml_dtypes.bfloat16)  # [nin, n]

    # ---- per-core edge buckets ----
    core = dst // per
    gidx = (dst % per) // g
    widx = src // win
    lid = (dst % per) % g

    # counts[c, g, w]
    counts = np.zeros((nc_, ng, nwin), dtype=np.int64)
    np.add.at(counts, (core, gidx, widx), 1)
    ncols_gw = np.ceil(counts / g).max(axis=0).astype(np.int64)  # [ng, nwin]

    # chunk layout: for K: for w: for g in K: ncols_gw[g, w] columns
    chunks = [
        list(range(k0, min(k0 + cfg.chunk_g, ng))) for k0 in range(0, ng, cfg.chunk_g)
    ]
    # stream metadata
    call_meta = []  # per (K, w): (slot_off, n_slots, [(gid, ncols), ...])
    region_off = {}  # (g, w) -> slot offset of its region
    off = 0
    for K in chunks:
        for w in range(nwin):
            groups = [(gg, int(ncols_gw[gg, w])) for gg in K if ncols_gw[gg, w] > 0]
            sl0 = off
        # register region offsets
            for gg, ncol in groups:
                region_off[(gg, w)] = off
                off += ncol * g
            call_meta.append((sl0, off - sl0, groups))
    tot_slots = off
    tot_cols = tot_slots // g

    # ---- per-core idx / localid arrays ----
    order = np.lexsort((src, widx, gidx, core))  # sort by core, g, w, src
    src_s, core_s = src[order], core[order]
    g_s, w_s, lid_s = gidx[order], widx[order], lid[order]

    idx_all = np.zeros((nc_, tot_slots), dtype=np.int16)
    lid_all = np.full((nc_, tot_slots), 255.0, dtype=np.float32)
    for c in range(nc_):
        m = core_s == c
        sc, gc, wc, lc = src_s[m], g_s[m], w_s[m], lid_s[m]
        # slot position: region_off[(g, w)] + rank within (g, w)
        # compute rank within each (g,w) run (data is sorted by (g,w))
        key = gc * nwin + wc
        # run-start indices
        change = np.r_[True, key[1:] != key[:-1]]
        run_id = np.cumsum(change) - 1
        run_start = np.flatnonzero(change)
        rank = np.arange(len(key)) - run_start[run_id]
        base = np.array([region_off[(gg, ww)] for gg, ww in zip(gc[change], wc[change])])
        slot = base[run_id] + rank
        idx_all[c, slot] = (sc - wc * win).astype(np.int16)
        lid_all[c, slot] = lc

    # wrap idx into [128, tot_slots//16] (16-partition wrap, replicated x8)
    idx_wrap = np.zeros((nc_, 128, tot_slots // 16), dtype=np.int16)
    lid_cols = np.zeros((nc_, 128, tot_cols), dtype=ml_dtypes.bfloat16)
    for c in range(nc_):
        wrapped = idx_all[c].reshape(-1, 16).T  # [16, S/16]
        idx_wrap[c] = np.tile(wrapped, (8, 1))
        lid_cols[c] = lid_all[c].reshape(tot_cols, g).T.astype(ml_dtypes.bfloat16)

    # ---- small constants ----
    iota = np.tile(np.arange(g, dtype=np.float32), (g, 1)).astype(ml_dtypes.bfloat16)
    W1bf = W1.astype(ml_dtypes.bfloat16)  # [nin, hid]
    W2p = np.zeros((cfg.hid, cfg.hid), dtype=np.float32)
    W2p[:, : cfg.outf] = W2
    W2bf = W2p.astype(ml_dtypes.bfloat16)
    b1col = b1.reshape(cfg.hid, 1).astype(np.float32)
    b2bc = np.zeros((128, cfg.hid), dtype=np.float32)
    b2bc[:, : cfg.outf] = b2[None, :]

    dinv_bc = np.zeros((nc_, cfg.hid, cfg.perp), dtype=np.float32)
    dinv_col = np.zeros((nc_, 128, ng), dtype=np.float32)
    for c in range(nc_):
        dslice = np.concatenate(
            [dinv[c * per : (c + 1) * per], np.ones(cfg.perp - per, np.float32)]
        )
        dinv_bc[c] = np.tile(dslice, (cfg.hid, 1))
        dinv_col[c] = dslice.reshape(ng, g).T

    in_maps = []
    for c in range(nc_):
        in_maps.append(
            {
                "xsT": np.asarray(xsT),
                "W1bf": np.asarray(W1bf),
                "W2bf": np.asarray(W2bf),
                "b1col": b1col,
                "b2bc": b2bc,
                "iota": np.asarray(iota),
                "idx": idx_wrap[c],
                "lid": np.asarray(lid_cols[c]),
                "dinv_bc": dinv_bc[c],
                "dinv_col": dinv_col[c],
            }
        )

    sched = dict(call_meta=call_meta, chunks=chunks, ncols_gw=ncols_gw,
                 tot_slots=tot_slots, tot_cols=tot_cols)
    return in_maps, sched


# ---------------------------------------------------------------- device side


def _build(cfg: Cfg, sched) -> bacc.Bacc:
    n, hid, g, nwin, win = cfg.n, cfg.hid, cfg.g, cfg.nwin, cfg.win
    ng, per, perp = cfg.ng, cfg.per, cfg.perp
    call_meta, chunks = sched["call_meta"], sched["chunks"]
    tot_slots, tot_cols = sched["tot_slots"], sched["tot_cols"]
    ncols_gw = sched["ncols_gw"]
    max_call_cols = max((m[1] // g for m in call_meta), default=1)

    nc = bacc.Bacc("TRN2", target_bir_lowering=False, debug=False,
                   num_devices=cfg.ncores)

    xsT = nc.dram_tensor("xsT", [cfg.nin, n], BF16, kind="ExternalInput").ap()
    W1bf = nc.dram_tensor("W1bf", [cfg.nin, hid], BF16, kind="ExternalInput").ap()
    W2bf = nc.dram_tensor("W2bf", [hid, hid], BF16, kind="ExternalInput").ap()
    b1col = nc.dram_tensor("b1col", [hid, 1], F32, kind="ExternalInput").ap()
    b2bc = nc.dram_tensor("b2bc", [128, hid], F32, kind="ExternalInput").ap()
    iota_d = nc.dram_tensor("iota", [g, g], BF16, kind="ExternalInput").ap()
    idx_d = nc.dram_tensor("idx", [128, tot_slots // 16], I16, kind="ExternalInput").ap()
    lid_d = nc.dram_tensor("lid", [128, tot_cols], BF16, kind="ExternalInput").ap()
    dinv_bc_d = nc.dram_tensor("dinv_bc", [hid, perp], F32, kind="ExternalInput").ap()
    dinv_col_d = nc.dram_tensor("dinv_col", [128, ng], F32, kind="ExternalInput").ap()

    out_d = nc.dram_tensor("out", [per, cfg.outf], F32, kind="ExternalOutput").ap()

    # Tables are [n, 2*hid] bf16: first hid cols hold values, upper half is
    # never read (dma_gather needs 256B rows; the pad halves are garbage).
    T1 = nc.dram_tensor("T1", [n, 2 * hid], BF16).ap()
    # Compact AllGather: ship only the outf real output columns, then expand
    # locally into the 256B-row gather table (pad columns are never read).
    of_ = cfg.outf
    h2c_b = nc.dram_tensor("h2c_b", [per, of_], BF16).ap()         # AG input bounce
    T3c = nc.dram_tensor("T3c", [n, of_], BF16, addr_space="Shared").ap()
    T3 = nc.dram_tensor("T3", [n, 2 * hid], BF16).ap()             # expanded table

    from concourse import library_config

    with tile.TileContext(nc) as tc, ExitStack() as ctx:
        nc.gpsimd.load_library(library_config.mlp)

        consts = ctx.enter_context(tc.tile_pool(name="consts", bufs=1))
        sb = ctx.enter_context(tc.tile_pool(name="sb", bufs=3))
        subp = ctx.enter_context(tc.tile_pool(name="subp", bufs=3))
        eptmp = ctx.enter_context(tc.tile_pool(name="eptmp", bufs=4))
        psum_bank = ctx.enter_context(tc.tile_pool(name="psumb", bufs=6, space="PSUM"))
        psum_mm = ctx.enter_context(tc.tile_pool(name="psummm", bufs=2, space="PSUM"))

        # resident constants
        w1_t = consts.tile([cfg.nin, hid], BF16)
        nc.sync.dma_start(w1_t[:], W1bf[:, :])
        w2_t = consts.tile([hid, hid], BF16)
        nc.sync.dma_start(w2_t[:], W2bf[:, :])
        b1_t = consts.tile([hid, 1], F32)
        nc.sync.dma_start(b1_t[:], b1col[:, :])
        b2_t = consts.tile([128, hid], F32)
        nc.sync.dma_start(b2_t[:], b2bc[:, :])
        iota_t = consts.tile([g, g], BF16)
        nc.sync.dma_start(iota_t[:], iota_d[:, :])
        # dense repeated iota [128, sub*g] so the S-build tensor_tensor has a
        # dense stride-1 first operand (DVE 2x eligibility)
        iota_rep = consts.tile([g, cfg.sub * g], BF16)
        for _s in range(cfg.sub):
            nc.vector.tensor_copy(iota_rep[:, _s * g : (_s + 1) * g], iota_t[:])
        dinvbc_t = consts.tile([hid, perp], F32)
        nc.sync.dma_start(dinvbc_t[:], dinv_bc_d[:, :])
        dinvcol_t = consts.tile([128, ng], F32)
        nc.sync.dma_start(dinvcol_t[:], dinv_col_d[:, :])
        t2t = consts.tile([hid, perp], BF16)  # T2^T staging (layer-1 output)
        # resident localid stream (shared by both layers); idx slices are
        # DMA'd per call (residency would cost 36KB/partition of SBUF)
        lid_t = consts.tile([128, tot_cols], BF16)
        nc.sync.dma_start(lid_t[:], lid_d[:, :])

        # -------------------------------------------------- T1 build
        # xchunk nodes per iteration: one input DMA, xchunk/128 matmuls into
        # psum banks of 8 x [128, 64], bank-wise ACT evictions, one fused 3D
        # output DMA (DRAM view [nsub, 128, hid] <- SBUF [128, nsub, hid]).
        xc = cfg.xchunk
        for ch0 in range(0, n, xc):
            cols = min(xc, n - ch0)
            nsub = math.ceil(cols / 128)
            xt = sb.tile([cfg.nin, xc], BF16, tag="xt")
            nc.sync.dma_start(xt[:, :cols], xsT[:, ch0 : ch0 + cols])
            st = sb.tile([128, xc // 128, hid], BF16, tag="t1s")
            for b0 in range(0, nsub, 8):
                bn = min(8, nsub - b0)
                pt = psum_mm.tile([128, 512], F32, tag="t1p", name="pt")
                for qi in range(bn):
                    q = b0 + qi
                    qc = min(128, cols - q * 128)
                    nc.tensor.matmul(
                        out=pt[:qc, qi * hid : qi * hid + hid],
                        lhsT=xt[:, q * 128 : q * 128 + qc],
                        rhs=w1_t[:],
                        start=True,
                        stop=True,
                    )
                nc.scalar.copy(
                    st[:, b0 : b0 + bn, :].rearrange("p q h -> p (q h)"),
                    pt[:, : bn * hid],
                )
            # fused transposed store into the value halves of T1 rows
            if cols % 128 == 0:
                dram_view = T1[ch0 : ch0 + cols, :hid].rearrange(
                    "(q p) h -> p q h", p=128
                )
                nc.sync.dma_start(dram_view, st[:, :nsub, :])
            else:
                for q in range(nsub):
                    qc = min(128, cols - q * 128)
                    nc.sync.dma_start(
                        T1[ch0 + q * 128 : ch0 + q * 128 + qc, :hid],
                        st[:qc, q, :],
                    )

        # -------------------------------------------------- aggregation layers
        def agg_layer(layer: int, table_ap):
            """layer 1: feature-major accum [hid, 128]; writes t2t + h2s.
            layer 2: node-major accum [128, hid]; writes log_softmax to out."""
            gper = 4 if layer == 1 else 8  # accumulator regions per PSUM bank
            ci = 0
            for K in chunks:
                # per-group accumulator sub-regions inside full-bank tiles
                nbank = math.ceil(len(K) / gper)
                banks = [
                    psum_bank.tile([128, 512], F32, tag="acc", name=f"acc{layer}")
                    for _ in range(nbank)
                ]

                def acc_ap(j):
                    b = banks[j // gper]
                    if layer == 1:
                        r = j % 4
                        return b[0:64, r * 128 : r * 128 + 128]
                    r = j % 8
                    return b[:, r * 64 : r * 64 + 64]

                # start/stop are BANK-granular: start=True clears has_written
                # for the whole bank, so only the first matmul into each bank
                # tile uses start=True and only the last uses stop=True;
                # per-region init relies on per-element overwrite semantics.
                tot_bank = [0] * nbank
                for j, gg in enumerate(K):
                    tot_bank[j // gper] += int(
                        sum(ncols_gw[gg, w] for w in range(nwin))
                    )
                emitted_bank = [0] * nbank
                for w in range(nwin):
                    sl0, nsl, groups = call_meta[ci]
                    ci += 1
                    if nsl == 0:
                        continue
                    cols = nsl // g
                    it = sb.tile([128, max_call_cols * 8], I16, tag="idx")
                    nc.sync.dma_start(
                        it[:, : nsl // 16], idx_d[:, sl0 // 16 : (sl0 + nsl) // 16]
                    )
                    mt = sb.tile([128, max_call_cols, 2 * hid], BF16, tag="m")
                    nc.gpsimd.dma_gather(
                        mt[:, :cols, :],
                        table_ap[w * win : (w + 1) * win, :],
                        it[:, : nsl // 16],
                        nsl,
                        nsl,
                        2 * hid,
                        single_packet=False,
                    )
                    # column -> (group-in-K index) map
                    colg = []
                    for gg, ncol in groups:
                        colg += [K.index(gg)] * ncol
                    for s0 in range(0, cols, cfg.sub):
                        sc = min(cfg.sub, cols - s0)
                        c0 = sl0 // g + s0
                        st_ = subp.tile([128, cfg.sub * g], BF16, tag="sel")
                        nc.vector.tensor_tensor(
                            out=st_[:, : sc * g].rearrange("p (c r) -> p c r", r=g),
                            in0=iota_rep[:, : sc * g].rearrange(
                                "p (c r) -> p c r", r=g
                            ),
                            in1=lid_t[:, c0 : c0 + sc].to_broadcast([128, sc, g]),
                            op=ALU.is_equal,
                        )
                        for j in range(sc):
                            gj = colg[s0 + j]
                            b = gj // gper
                            first = emitted_bank[b] == 0
                            emitted_bank[b] += 1
                            last = emitted_bank[b] == tot_bank[b]
                            if layer == 1:
                                nc.tensor.matmul(
                                    out=acc_ap(gj),
                                    lhsT=mt[:, s0 + j, :hid],
                                    rhs=st_[:, j * g : (j + 1) * g],
                                    start=first,
                                    stop=last,
                                )
                            else:
                                nc.tensor.matmul(
                                    out=acc_ap(gj),
                                    lhsT=st_[:, j * g : (j + 1) * g],
                                    rhs=mt[:, s0 + j, :hid],
                                    start=first,
                                    stop=last,
                                )
                # epilogues for chunk K
                if layer == 2:
                    ot_stage = sb.tile(
                        [128, cfg.chunk_g, cfg.outf], F32, tag="otst", name="ot_stage"
                    )
                for j, gg in enumerate(K):
                    rows = min(g, per - gg * g)  # real dst rows in group
                    if layer == 1:
                        dslice = dinvbc_t[:, gg * g : gg * g + g]
                        t1_ = eptmp.tile([hid, g], F32, tag="ep1")
                        nc.vector.tensor_mul(t1_[:], acc_ap(j)[:], dslice[:])
                        t2_ = eptmp.tile([hid, g], F32, tag="ep2")
                        nc.scalar.activation(t2_[:], t1_[:], AF.Relu, bias=b1_t[:, :1])
                        nc.vector.tensor_mul(
                            t2t[:, gg * g : gg * g + g], t2_[:], dslice[:]
                        )
                    else:
                        t1_ = eptmp.tile([128, hid], F32, tag="ep1")
                        nc.vector.tensor_scalar(
                            t1_[:], acc_ap(j)[:], dinvcol_t[:, gg : gg + 1], None,
                            ALU.mult,
                        )
                        t2_ = eptmp.tile([128, hid], F32, tag="ep2")
                        nc.vector.tensor_add(t2_[:], t1_[:], b2_t[:])
                        of = cfg.outf
                        nmax = eptmp.tile([128, 1], F32, tag="nmax")
                        nc.vector.tensor_reduce(
                            nmax[:], t2_[:, :of], mybir.AxisListType.X, ALU.max,
                            negate=True,
                        )
                        ex = eptmp.tile([128, of], F32, tag="ex")
                        nc.scalar.activation(ex[:], t2_[:, :of], AF.Exp, bias=nmax[:, :1])
                        sm = eptmp.tile([128, 1], F32, tag="sm")
                        nc.vector.tensor_reduce(
                            sm[:], ex[:], mybir.AxisListType.X, ALU.add
                        )
                        ls = eptmp.tile([128, 1], F32, tag="ls")
                        nc.scalar.activation(ls[:], sm[:], AF.Ln)
                        nc.vector.tensor_scalar(
                            ot_stage[:, j, :], t2_[:, :of], nmax[:, :1], ls[:, :1],
                            ALU.add, ALU.subtract,
                        )
                if layer == 2:
                    # one fused store for the chunk's full groups, small store
                    # for a trailing partial group
                    nfull = sum(1 for gg in K if per - gg * g >= g)
                    r0 = K[0] * g
                    if nfull:
                        nc.sync.dma_start(
                            out_d[r0 : r0 + nfull * g, :].rearrange(
                                "(q p) f -> p q f", p=128
                            ),
                            ot_stage[:, :nfull, :],
                        )
                    for j, gg in enumerate(K):
                        rows = per - gg * g
                        if rows < g:
                            nc.sync.dma_start(
                                out_d[gg * g : gg * g + rows, :],
                                ot_stage[:rows, j, :],
                            )

        _phases = int(os.environ.get("GCN_PHASES", "4"))  # 1=t1 2=+l1 3=+ag 4=all
        if _phases >= 2:
            agg_layer(1, T1)

        # -------------------------------------------------- W2 matmul + AllGather
        if _phases >= 3:
            for g0 in range(0, ng, 8):
                gn = min(8, ng - g0)
                pw = psum_mm.tile([128, 512], F32, tag="t1p", name="pw")
                for qi in range(gn):
                    gg = g0 + qi
                    nc.tensor.matmul(
                        out=pw[:, qi * hid : qi * hid + hid],
                        lhsT=t2t[:, gg * g : gg * g + g],
                        rhs=w2_t[:],
                        start=True,
                        stop=True,
                    )
                hw = sb.tile([128, 8, hid], BF16, tag="h2t")
                nc.scalar.copy(
                    hw[:, :gn, :].rearrange("p q h -> p (q h)"), pw[:, : gn * hid]
                )
                nfull = sum(1 for gg in range(g0, g0 + gn) if per - gg * g >= g)
                if nfull:
                    nc.sync.dma_start(
                        h2c_b[g0 * g : (g0 + nfull) * g, :].rearrange(
                            "(q p) h -> p q h", p=128
                        ),
                        hw[:, :nfull, :of_],
                    )
                for qi in range(gn):
                    gg = g0 + qi
                    rows = per - gg * g
                    if rows < g:
                        nc.sync.dma_start(
                            h2c_b[gg * g : gg * g + rows, :], hw[:rows, qi, :of_]
                        )

        if _phases >= 3 and not os.environ.get("GCN_NO_COLL"):
            nc.gpsimd.collective_compute(
                "AllGather",
                ALU.bypass,
                replica_groups=[list(range(cfg.ncores))],
                ins=[h2c_b.opt()],
                outs=[T3c.opt()],
            )
            # expand compact rows into the 256B-row gather table; the pad
            # columns of T3 stay uninitialized and are never read.
            for r0 in range(0, n, per):
                nc.sync.dma_start(T3[r0 : r0 + per, :of_], T3c[r0 : r0 + per, :])

        if _phases >= 4:
            agg_layer(2, T3)

    nc.compile()
    return nc


# ---------------------------------------------------------------- entry


def kernel(x, edge_index, W1, b1, W2, b2, cfg: Cfg | None = None, _run=None):
    cfg = cfg or Cfg()
    in_maps, sched = _preprocess(
        np.asarray(x), np.asarray(edge_index), np.asarray(W1), np.asarray(b1),
        np.asarray(W2), np.asarray(b2), cfg
    )
    nc = _build(cfg, sched)
    if _run is not None:  # test hook (e.g. simulator)
        results = _run(nc, in_maps)
    else:
        results = run_bass_kernel_spmd(
            nc, in_maps, core_ids=list(range(cfg.ncores))
        ).results
    out = np.concatenate([results[c]["out"] for c in range(cfg.ncores)], axis=0)
    return out.astype(np.float32)
